# revision 4
# baseline (speedup 1.0000x reference)
"""GAT (2-layer DGL GATConv) on 8 TRN2 NeuronCores — single-NEFF design.

Key facts driving the design (measured on HW):
- SWDGE indirect-gather descriptor GENERATION is serial: ~1.29us per
  128-row gather, independent of row size and queue count. One gather
  per 128-edge tile per layer is therefore the critical path (~1ms per
  layer); every other engine is scheduled to hide under it.
- gpsimd local_scatter adds ~132ns to the same serial unit, so the
  [slot,edge] one-hot is built OFF Pool: S_t via DVE iota+is_equal(ptr),
  its transpose via PE+copy.

Structure: nodes LPT-packed into 392 balanced dst-blocks (49/core);
tables in GLOBAL node order (no per-core rotation). er1 per own block
from a small matmul over the core's own feature shard (straight to
SBUF). z = ST^T@er_blk + I@el via PE; leaky+exp on Act; msg scaling on
DVE. Layer-2 node transform fused into layer-1 evacuation; table2
shards AllGathered in-kernel; L2 one-hot carries the edge weight as its
value (iota is_equal*w ptr) so L2 has no per-edge multiply. Residual in
SBUF. Single launch, no host round trip.
"""
import sys
sys.path.insert(0, '/opt/trn_rl_repo')
import numpy as np
import ml_dtypes
from contextlib import ExitStack

import concourse.bass as bass
import concourse.tile as tile
from concourse import mybir, bacc
from concourse.bass_interp import MultiCoreSim, get_hw_module

bf16 = ml_dtypes.bfloat16
P = 128
NC = 8
N = 50000
IN_CH = 256
H1, D1 = 4, 64
NEG = 0.2
NB = 49
NPC = NB * P            # 6272
NROT = NC * NPC         # 50176
NBLK_G = NROT // P      # 392
ROW1 = 264              # table1: ft(256)|el(4)|er(4)
ROW2 = 66               # table2: ft2(64)|el2|er2
CH = 8
AGC = 4               # AllGather chunk (blocks)

_timing = {}


def _finalize(nc, n_cores=NC):
    nc.compile()
    MultiCoreSim(nc, num_cores=n_cores, trace=False)
    nc.m = get_hw_module(nc.m)
    return nc


def _prepare(nc, in_maps, n_cores=NC):
    """Replicates bass2jax.run_bass_via_pjrt with device-resident inputs and
    no donation so the callable can be re-run for timing."""
    import jax
    from jax.sharding import Mesh, PartitionSpec, NamedSharding
    from jax.experimental.shard_map import shard_map
    from concourse import bass2jax
    from concourse.bass2jax import _bass_exec_p, install_neuronx_cc_hook

    install_neuronx_cc_hook()
    partition_name = nc.partition_id_tensor.name if nc.partition_id_tensor else None
    in_names, out_names, out_avals, zero_outs = [], [], [], []
    for alloc in nc.m.functions[0].allocations:
        if not isinstance(alloc, mybir.MemoryLocationSet):
            continue
        name = alloc.memorylocations[0].name
        if alloc.kind == "ExternalInput":
            if name != partition_name:
                in_names.append(name)
        elif alloc.kind == "ExternalOutput":
            shape = tuple(alloc.tensor_shape)
            dtype = mybir.dt.np(alloc.dtype)
            out_names.append(name)
            out_avals.append(jax.core.ShapedArray(shape, dtype))
            zero_outs.append(np.zeros(shape, dtype))
    n_params = len(in_names)
    all_in = list(in_names) + list(out_names)
    if partition_name is not None:
        all_in.append(partition_name)

    def _body(*args):
        operands = list(args)
        if partition_name is not None:
            operands.append(bass2jax.partition_id_tensor())
        return tuple(_bass_exec_p.bind(
            *operands, out_avals=tuple(out_avals), in_names=tuple(all_in),
            out_names=tuple(out_names), lowering_input_output_aliases=(),
            sim_require_finite=True, sim_require_nnan=True, nc=nc))

    devices = jax.devices()[:n_cores]
    mesh = Mesh(np.asarray(devices), ("core",))
    specs_in = (PartitionSpec("core"),) * (n_params + len(out_names))
    specs_out = (PartitionSpec("core"),) * len(out_names)
    fn = jax.jit(shard_map(_body, mesh=mesh, in_specs=specs_in,
                           out_specs=specs_out, check_rep=False),
                 keep_unused=True)
    per_core = [[np.asarray(m[name]) for name in in_names] for m in in_maps]
    concat_in = [np.concatenate([per_core[c][i] for c in range(n_cores)], axis=0)
                 for i in range(n_params)]
    concat_z = [np.zeros((n_cores * z.shape[0], *z.shape[1:]), z.dtype)
                for z in zero_outs]
    shard = NamedSharding(mesh, PartitionSpec("core"))
    dev_in = [jax.device_put(a, shard) for a in concat_in]
    dev_z = [jax.device_put(a, shard) for a in concat_z]

    def run_fn():
        outs = fn(*dev_in, *dev_z)
        jax.block_until_ready(outs)
        return [{name: np.asarray(outs[i]).reshape(n_cores, *out_avals[i].shape)[c]
                 for i, name in enumerate(out_names)}
                for c in range(n_cores)], outs

    def time_fn(iters=8, warmup=2):
        import time as _time
        for _ in range(warmup):
            jax.block_until_ready(fn(*dev_in, *dev_z))
        ts = []
        for _ in range(iters):
            t0 = _time.perf_counter()
            jax.block_until_ready(fn(*dev_in, *dev_z))
            ts.append(_time.perf_counter() - t0)
        return min(ts)

    run_fn.time_fn = time_fn
    return run_fn

# ---------------------------------------------------------------- host prep

def _host_prep(feat, src, dst, W1, al1, ar1, W2, al2, ar2, resW2):
    import heapq
    feat = np.asarray(feat, np.float32)
    src = np.asarray(src).astype(np.int64)
    dst = np.asarray(dst).astype(np.int64)
    W1 = np.asarray(W1, np.float64)
    W2 = np.asarray(W2, np.float64)
    al1 = np.asarray(al1, np.float64)
    ar1 = np.asarray(ar1, np.float64)
    al2 = np.asarray(al2, np.float64)
    ar2 = np.asarray(ar2, np.float64)
    resW2 = np.asarray(resW2, np.float64)

    # balanced node->block assignment (LPT by in-degree, cap 128/bin)
    deg = np.bincount(dst, minlength=N)
    order = np.argsort(-deg, kind='stable')
    heap = [(0, b) for b in range(NBLK_G)]
    heapq.heapify(heap)
    bin_cnt = np.zeros(NBLK_G, np.int32)
    newid = np.empty(N, np.int64)
    for n in order:
        load, b = heapq.heappop(heap)
        newid[n] = b * P + bin_cnt[b]
        bin_cnt[b] += 1
        if bin_cnt[b] < P:
            heapq.heappush(heap, (load + int(deg[n]), b))

    featp = np.zeros((NROT, IN_CH), np.float32)
    featp[newid] = feat
    featT = np.ascontiguousarray(featp.T).astype(bf16)          # [256, NROT]

    Wl1 = np.stack([W1[:, h*D1:(h+1)*D1] @ al1[h] for h in range(H1)], axis=1)
    Wr1 = np.stack([W1[:, h*D1:(h+1)*D1] @ ar1[h] for h in range(H1)], axis=1)
    W1ext = np.concatenate([W1, Wl1, Wr1], axis=1).astype(bf16)       # [256,264]
    Wl2 = (W2 @ al2[0])[:, None]
    Wr2 = (W2 @ ar2[0])[:, None]
    W2R = np.concatenate([W2, Wl2, Wr2, resW2], axis=1).astype(bf16)  # [256,130]
    Wr1b = Wr1.astype(bf16)                                           # [256,4]

    src_n = newid[src]
    dst_n = newid[dst]
    core_e = dst_n // NPC
    cnt = np.zeros((NC, NB), np.int64)
    percore = []
    for c in range(NC):
        m = core_e == c
        es = src_n[m]
        loc = dst_n[m] - c * NPC
        blk = loc >> 7
        slot = loc & 127
        for b in range(NB):
            cnt[c, b] = np.count_nonzero(blk == b)
        percore.append((es, blk, slot))
    TB = np.maximum(1, -(-cnt.max(axis=0) // P)).astype(np.int64)     # [NB]
    T = int(TB.sum())
    toff = np.zeros(NB + 1, np.int64)
    toff[1:] = np.cumsum(TB)

    in_maps = []
    for c in range(NC):
        es, blk, slot = percore[c]
        src_idx = np.zeros((T, P), np.int32)
        slotf = np.full((T, P), -1.0, np.float32)
        for b in range(NB):
            bm = blk == b
            e_s = es[bm]
            e_sl = slot[bm]
            n = len(e_s)
            nslots = TB[b] * P
            pad_s = np.zeros(nslots, np.int64)
            pad_s[:n] = e_s
            pad_f = np.full(nslots, -1.0, np.float32)
            pad_f[:n] = e_sl
            src_idx[toff[b]:toff[b+1]] = pad_s.reshape(TB[b], P)
            slotf[toff[b]:toff[b+1]] = pad_f.reshape(TB[b], P)
        # chunk-major remap for the chunked-AllGather table2 layout
        CHP = AGC * P
        sizes = [min(CHP, NPC - k*CHP) for k in range((NPC + CHP - 1)//CHP)]
        bases = np.cumsum([0] + [NC * s for s in sizes[:-1]])
        gi = src_idx.astype(np.int64)
        cc = gi // NPC
        nn = gi % NPC
        kk = nn // CHP
        sk = np.asarray(sizes, np.int64)[kk]
        src_idx2 = (np.asarray(bases, np.int64)[kk] + cc*sk + (nn - kk*CHP)).astype(np.int32)
        in_maps.append({
            "featT": featT,
            "fshT": np.ascontiguousarray(featT[:, c*NPC:(c+1)*NPC]),
            "W1ext": W1ext,
            "W2R": W2R,
            "Wr1b": Wr1b,
            "ident": np.eye(P, dtype=bf16),
            "src_idx": np.ascontiguousarray(src_idx.T),       # [P, T] int32
            "src_idx2": np.ascontiguousarray(src_idx2.T),     # [P, T] int32
            "slotf": np.ascontiguousarray(slotf.T),           # [P, T] f32
        })
    return in_maps, TB.tolist(), newid


# ---------------------------------------------------------------- kernel

def _build(TB):
    T = sum(TB)
    toff = [0]
    for t in TB:
        toff.append(toff[-1] + t)

    nc = bacc.Bacc("TRN2", target_bir_lowering=False, debug=False,
                   num_devices=NC, enable_asserts=False)
    dt = mybir.dt
    featT = nc.dram_tensor("featT", [IN_CH, NROT], dt.bfloat16, kind="ExternalInput").ap()
    fshT = nc.dram_tensor("fshT", [IN_CH, NPC], dt.bfloat16, kind="ExternalInput").ap()
    W1e = nc.dram_tensor("W1ext", [IN_CH, ROW1], dt.bfloat16, kind="ExternalInput").ap()
    W2R = nc.dram_tensor("W2R", [IN_CH, 130], dt.bfloat16, kind="ExternalInput").ap()
    Wr1b = nc.dram_tensor("Wr1b", [IN_CH, 4], dt.bfloat16, kind="ExternalInput").ap()
    ident = nc.dram_tensor("ident", [P, P], dt.bfloat16, kind="ExternalInput").ap()
    src_idx = nc.dram_tensor("src_idx", [P, T], dt.int32, kind="ExternalInput").ap()
    src_idx2 = nc.dram_tensor("src_idx2", [P, T], dt.int32, kind="ExternalInput").ap()
    slotf = nc.dram_tensor("slotf", [P, T], dt.float32, kind="ExternalInput").ap()

    table1 = nc.dram_tensor("table1", [NROT, ROW1], dt.bfloat16, kind="Internal").ap()
    t2shard = nc.dram_tensor("t2shard", [NPC, ROW2], dt.bfloat16, kind="Internal").ap()
    table2 = nc.dram_tensor("table2", [NROT, ROW2], dt.bfloat16, kind="Internal",
                            addr_space="Shared").ap()
    out = nc.dram_tensor("out_shard", [NPC, D1], dt.float32, kind="ExternalOutput").ap()

    AF = mybir.ActivationFunctionType
    ALU = mybir.AluOpType

    with tile.TileContext(nc) as tc, ExitStack() as ctx:
        cst = ctx.enter_context(tc.tile_pool(name="cst", bufs=1))
        W1e_t = cst.tile([P, 2, ROW1], dt.bfloat16)
        nc.sync.dma_start(W1e_t[:, 0, :], W1e[0:P, :])
        nc.sync.dma_start(W1e_t[:, 1, :], W1e[P:2*P, :])
        W2R_t = cst.tile([P, 2, 130], dt.bfloat16)
        nc.sync.dma_start(W2R_t[:, 0, :], W2R[0:P, :])
        nc.sync.dma_start(W2R_t[:, 1, :], W2R[P:2*P, :])
        Wr1_t = cst.tile([P, 2, 4], dt.bfloat16)
        nc.sync.dma_start(Wr1_t[:, 0, :], Wr1b[0:P, :])
        nc.sync.dma_start(Wr1_t[:, 1, :], Wr1b[P:2*P, :])
        ident_t = cst.tile([P, P], dt.bfloat16)
        nc.sync.dma_start(ident_t[:], ident[:, :])
        src_idx_t = cst.tile([P, T], dt.int32)
        nc.sync.dma_start(src_idx_t[:], src_idx[:, :])
        src_idx2_t = cst.tile([P, T], dt.int32)
        nc.sync.dma_start(src_idx2_t[:], src_idx2[:, :])
        slotf_t = cst.tile([P, T], dt.float32)
        nc.sync.dma_start(slotf_t[:], slotf[:, :])
        iota_t = cst.tile([P, P], dt.bfloat16)
        nc.gpsimd.iota(iota_t[:], pattern=[[1, P]], base=0, channel_multiplier=0,
                       allow_small_or_imprecise_dtypes=True)
        er1_sb = cst.tile([P, NB, 4], dt.bfloat16)
        er2_sb = cst.tile([P, NB], dt.bfloat16)
        res_sb = cst.tile([P, NB, D1], dt.float32)

        # ---------------- er-shard pass: er1_sb[:, b, :] for own blocks
        with ExitStack() as ectx:
            e_sb = ectx.enter_context(tc.tile_pool(name="e_sb", bufs=2))
            e_ps = ectx.enter_context(tc.tile_pool(name="e_ps", bufs=4, space="PSUM"))
            nchunks = (NB + CH - 1) // CH
            for ch in range(nchunks):
                b0 = ch * CH
                nb_in = min(CH, NB - b0)
                w = nb_in * P
                lhs = e_sb.tile([P, 2, CH * P], dt.bfloat16, tag="lhs")
                nc.sync.dma_start(lhs[:, 0, 0:w], fshT[0:P, b0*P:b0*P+w])
                nc.sync.dma_start(lhs[:, 1, 0:w], fshT[P:2*P, b0*P:b0*P+w])
                for j in range(nb_in):
                    ps = e_ps.tile([P, 4], dt.float32, space="PSUM", tag="ps")
                    nc.tensor.matmul(ps[:], lhsT=lhs[:, 0, j*P:(j+1)*P],
                                     rhs=Wr1_t[:, 0, :], start=True, stop=False)
                    nc.tensor.matmul(ps[:], lhsT=lhs[:, 1, j*P:(j+1)*P],
                                     rhs=Wr1_t[:, 1, :], start=False, stop=True)
                    nc.vector.tensor_copy(er1_sb[:, b0 + j, :], ps[:])

        # ---------------- node phase: table1 = [feat@W1 | el | er] (global)
        with ExitStack() as nctx:
            np_sb = nctx.enter_context(tc.tile_pool(name="np_sb", bufs=3))
            np_out = nctx.enter_context(tc.tile_pool(name="np_out", bufs=3))
            np_ps = nctx.enter_context(tc.tile_pool(name="np_ps", bufs=8, space="PSUM"))
            for ch in range(NBLK_G // CH):
                lhs = np_sb.tile([P, 2, CH * P], dt.bfloat16, tag="lhs")
                nc.sync.dma_start(lhs[:, 0, :], featT[0:P, ch*CH*P:(ch+1)*CH*P])
                nc.sync.dma_start(lhs[:, 1, :], featT[P:2*P, ch*CH*P:(ch+1)*CH*P])
                rows = np_out.tile([P, CH, ROW1], dt.bfloat16, tag="rows")
                for j in range(CH):
                    ps = np_ps.tile([P, ROW1], dt.float32, space="PSUM", tag="ps")
                    nc.tensor.matmul(ps[:], lhsT=lhs[:, 0, j*P:(j+1)*P],
                                     rhs=W1e_t[:, 0, :], start=True, stop=False)
                    nc.tensor.matmul(ps[:], lhsT=lhs[:, 1, j*P:(j+1)*P],
                                     rhs=W1e_t[:, 1, :], start=False, stop=True)
                    if j % 2 == 0:
                        nc.scalar.activation(rows[:, j, :], ps[:], AF.Copy)
                    else:
                        nc.vector.tensor_copy(rows[:, j, :], ps[:])
                nc.sync.dma_start(
                    table1[ch*CH*P:(ch+1)*CH*P, :].rearrange("(j p) r -> p j r", p=P),
                    rows[:])

        # ---------------- layer-1 edge phase
        with ExitStack() as ectx:
            g_pool = ectx.enter_context(tc.tile_pool(name="g1", bufs=10))
            s_pool = ectx.enter_context(tc.tile_pool(name="s1", bufs=8))
            st_pool = ectx.enter_context(tc.tile_pool(name="st1", bufs=8))
            zl_pool = ectx.enter_context(tc.tile_pool(name="zl1", bufs=8))
            msg_pool = ectx.enter_context(tc.tile_pool(name="m1", bufs=8))
            ev_pool = ectx.enter_context(tc.tile_pool(name="ev1", bufs=2))
            t2_pool = ectx.enter_context(tc.tile_pool(name="t2s", bufs=2))
            z_ps = ectx.enter_context(tc.tile_pool(name="z1p", bufs=1, space="PSUM"))
            tr_ps = ectx.enter_context(tc.tile_pool(name="tr1", bufs=2, space="PSUM"))
            agg_ps = ectx.enter_context(tc.tile_pool(name="agg1", bufs=2, space="PSUM"))
            l2_ps = ectx.enter_context(tc.tile_pool(name="l2n", bufs=1, space="PSUM"))

            t2st = None
            for b in range(NB):
                tb, t0 = TB[b], toff[b]
                agg = agg_ps.tile([P, 260], dt.float32, space="PSUM", tag="agg")
                for j in range(tb):
                    t = t0 + j
                    g = g_pool.tile([P, ROW1], dt.bfloat16, tag="g")
                    nc.gpsimd.indirect_dma_start(
                        out=g[:], out_offset=None, in_=table1[:, :],
                        in_offset=bass.IndirectOffsetOnAxis(
                            ap=src_idx_t[:, t:t+1], axis=0))
                    S_t = s_pool.tile([P, P], dt.bfloat16, tag="S")
                    nc.vector.tensor_scalar(out=S_t[:], in0=iota_t[:],
                                            scalar1=slotf_t[:, t:t+1], scalar2=None,
                                            op0=ALU.is_equal)
                    stp = tr_ps.tile([P, P], dt.bfloat16, space="PSUM", tag="stp")
                    nc.tensor.transpose(stp[:], S_t[:], ident_t[:])
                    ST_t = st_pool.tile([P, P], dt.bfloat16, tag="ST")
                    nc.scalar.activation(ST_t[:], stp[:], AF.Copy)
                    zps = z_ps.tile([P, 4], dt.float32, space="PSUM", tag="zps")
                    nc.tensor.matmul(zps[:], lhsT=ST_t[:], rhs=er1_sb[:, b, :],
                                     start=True, stop=False)
                    nc.tensor.matmul(zps[:], lhsT=ident_t[:], rhs=g[:, 256:260],
                                     start=False, stop=True)
                    msg = msg_pool.tile([P, 260], dt.bfloat16, tag="msg")
                    zc = zl_pool.tile([P, 4], dt.float32, tag="zc")
                    nc.vector.tensor_scalar(out=zc[:], in0=zps[:], scalar1=-300.0,
                                            scalar2=None, op0=ALU.max)
                    zl = zl_pool.tile([P, 4], dt.float32, tag="zl")
                    nc.vector.scalar_tensor_tensor(out=zl[:], in0=zc[:], scalar=NEG,
                                                   in1=zc[:], op0=ALU.mult, op1=ALU.max)
                    nc.scalar.activation(msg[:, 256:260], zl[:], AF.Exp)
                    nc.vector.tensor_tensor(
                        out=msg[:, 0:256].rearrange("p (h d) -> p h d", h=4),
                        in0=g[:, 0:256].rearrange("p (h d) -> p h d", h=4),
                        in1=msg[:, 256:260][:, :, None].to_broadcast([P, 4, D1]),
                        op=ALU.mult)
                    nc.tensor.matmul(agg[:], lhsT=S_t[:], rhs=msg[:],
                                     start=(j == 0), stop=(j == tb - 1))
                # ---- evacuate block: h1 = ELU(agg/den); fused L2 node xform
                dmax = ev_pool.tile([P, 4], dt.float32, tag="dmax")
                nc.vector.tensor_scalar(out=dmax[:], in0=agg[:, 256:260],
                                        scalar1=1e-30, scalar2=None, op0=ALU.max)
                recip = ev_pool.tile([P, 4], dt.float32, tag="recip")
                nc.vector.reciprocal(recip[:], dmax[:])
                rst = ev_pool.tile([P, 4, D1], dt.float32, tag="rst")
                nc.vector.tensor_tensor(
                    out=rst[:],
                    in0=agg[:, 0:256].rearrange("p (h d) -> p h d", h=4),
                    in1=recip[:, :, None].to_broadcast([P, 4, D1]), op=ALU.mult)
                rstf = rst[:].rearrange("p h d -> p (h d)")
                mn = ev_pool.tile([P, 256], dt.float32, tag="mn")
                nc.vector.tensor_scalar(out=mn[:], in0=rstf, scalar1=0.0,
                                        scalar2=None, op0=ALU.min)
                exm = ev_pool.tile([P, 256], dt.float32, tag="exm")
                nc.scalar.activation(exm[:], mn[:], AF.Exp)
                h1p = ev_pool.tile([P, 256], dt.float32, tag="h1p")
                nc.vector.scalar_tensor_tensor(out=h1p[:], in0=rstf, scalar=0.0,
                                               in1=exm[:], op0=ALU.max, op1=ALU.add)
                h1b = ev_pool.tile([P, 256], dt.bfloat16, tag="h1b")
                nc.vector.tensor_scalar(out=h1b[:], in0=h1p[:], scalar1=-1.0,
                                        scalar2=None, op0=ALU.add)
                h1T = ev_pool.tile([P, 2, P], dt.bfloat16, tag="h1T")
                for half in range(2):
                    ptr = tr_ps.tile([P, P], dt.bfloat16, space="PSUM", tag="ptr")
                    nc.tensor.transpose(ptr[:], h1b[:, half*P:(half+1)*P], ident_t[:])
                    if half == 0:
                        nc.scalar.activation(h1T[:, half, :], ptr[:], AF.Copy)
                    else:
                        nc.vector.tensor_copy(h1T[:, half, :], ptr[:])
                ps2 = l2_ps.tile([P, 130], dt.float32, space="PSUM", tag="ps2")
                nc.tensor.matmul(ps2[:], lhsT=h1T[:, 0, :], rhs=W2R_t[:, 0, :],
                                 start=True, stop=False)
                nc.tensor.matmul(ps2[:], lhsT=h1T[:, 1, :], rhs=W2R_t[:, 1, :],
                                 start=False, stop=True)
                if b % AGC == 0:
                    t2st = t2_pool.tile([P, AGC, ROW2], dt.bfloat16, tag="t2st")
                nc.scalar.activation(t2st[:, b % AGC, :], ps2[:, 0:ROW2], AF.Copy)
                nc.vector.tensor_copy(er2_sb[:, b:b+1], t2st[:, b % AGC, 65:66])
                nc.vector.tensor_copy(res_sb[:, b, :], ps2[:, ROW2:130])
                if b % AGC == AGC - 1 or b == NB - 1:
                    b0 = (b // AGC) * AGC
                    nb_in = b - b0 + 1
                    nc.sync.dma_start(
                        t2shard[b0*P:(b+1)*P, :].rearrange("(j p) r -> p j r", p=P),
                        t2st[:, 0:nb_in, :])
                    # AllGather this chunk (chunk-major contiguous output)
                    sk = nb_in * P
                    base = NC * b0 * P
                    nc.gpsimd.collective_compute(
                        "AllGather", ALU.bypass,
                        replica_groups=[list(range(NC))],
                        ins=[t2shard[b0*P:(b+1)*P, :].opt()],
                        outs=[table2[base:base + NC*sk, :].opt()])

        # ---------------- layer-2 edge phase
        with ExitStack() as ectx:
            g_pool = ectx.enter_context(tc.tile_pool(name="g2", bufs=10))
            s_pool = ectx.enter_context(tc.tile_pool(name="s2", bufs=8))
            st_pool = ectx.enter_context(tc.tile_pool(name="st2", bufs=8))
            w_pool = ectx.enter_context(tc.tile_pool(name="w2", bufs=8))
            ev_pool = ectx.enter_context(tc.tile_pool(name="ev2", bufs=2))
            o_pool = ectx.enter_context(tc.tile_pool(name="o2", bufs=2))
            z_ps = ectx.enter_context(tc.tile_pool(name="z2p", bufs=1, space="PSUM"))
            tr_ps = ectx.enter_context(tc.tile_pool(name="tr2", bufs=2, space="PSUM"))
            agg_ps = ectx.enter_context(tc.tile_pool(name="agg2", bufs=2, space="PSUM"))

            ost = None
            for b in range(NB):
                tb, t0 = TB[b], toff[b]
                agg2 = agg_ps.tile([P, 65], dt.float32, space="PSUM", tag="agg")
                for j in range(tb):
                    t = t0 + j
                    g2 = g_pool.tile([P, ROW2], dt.bfloat16, tag="g")
                    nc.gpsimd.indirect_dma_start(
                        out=g2[:], out_offset=None, in_=table2[:, :],
                        in_offset=bass.IndirectOffsetOnAxis(
                            ap=src_idx2_t[:, t:t+1], axis=0))
                    S_t = s_pool.tile([P, P], dt.bfloat16, tag="S")
                    nc.vector.tensor_scalar(out=S_t[:], in0=iota_t[:],
                                            scalar1=slotf_t[:, t:t+1], scalar2=None,
                                            op0=ALU.is_equal)
                    stp = tr_ps.tile([P, P], dt.bfloat16, space="PSUM", tag="stp")
                    nc.tensor.transpose(stp[:], S_t[:], ident_t[:])
                    ST_t = st_pool.tile([P, P], dt.bfloat16, tag="ST")
                    nc.scalar.activation(ST_t[:], stp[:], AF.Copy)
                    zps = z_ps.tile([P, 1], dt.float32, space="PSUM", tag="zps")
                    nc.tensor.matmul(zps[:], lhsT=ST_t[:], rhs=er2_sb[:, b:b+1],
                                     start=True, stop=False)
                    nc.tensor.matmul(zps[:], lhsT=ident_t[:], rhs=g2[:, 64:65],
                                     start=False, stop=True)
                    zc = w_pool.tile([P, 1], dt.float32, tag="zc")
                    nc.vector.tensor_scalar(out=zc[:], in0=zps[:], scalar1=-300.0,
                                            scalar2=None, op0=ALU.max)
                    zl = w_pool.tile([P, 1], dt.float32, tag="zl")
                    nc.vector.scalar_tensor_tensor(out=zl[:], in0=zc[:], scalar=NEG,
                                                   in1=zc[:], op0=ALU.mult, op1=ALU.max)
                    w2 = w_pool.tile([P, 1], dt.float32, tag="w")
                    nc.scalar.activation(w2[:], zl[:], AF.Exp)
                    # denominator column: overwrite el2 with 1.0 (after z used it)
                    nc.vector.memset(g2[:, 64:65], 1.0)
                    S_w = s_pool.tile([P, P], dt.bfloat16, tag="Sw")
                    nc.vector.tensor_scalar(out=S_w[:], in0=iota_t[:],
                                            scalar1=slotf_t[:, t:t+1],
                                            scalar2=w2[:, 0:1],
                                            op0=ALU.is_equal, op1=ALU.mult)
                    nc.tensor.matmul(agg2[:], lhsT=S_w[:], rhs=g2[:, 0:65],
                                     start=(j == 0), stop=(j == tb - 1))
                dmax = ev_pool.tile([P, 1], dt.float32, tag="dmax")
                nc.vector.tensor_scalar(out=dmax[:], in0=agg2[:, 64:65],
                                        scalar1=1e-30, scalar2=None, op0=ALU.max)
                recip = ev_pool.tile([P, 1], dt.float32, tag="recip")
                nc.vector.reciprocal(recip[:], dmax[:])
                if b % CH == 0:
                    ost = o_pool.tile([P, CH, D1], dt.float32, tag="ost")
                nc.vector.scalar_tensor_tensor(out=ost[:, b % CH, :],
                                               in0=agg2[:, 0:D1],
                                               scalar=recip[:, 0:1],
                                               in1=res_sb[:, b, :],
                                               op0=ALU.mult, op1=ALU.add)
                if b % CH == CH - 1 or b == NB - 1:
                    b0 = (b // CH) * CH
                    nb_in = b - b0 + 1
                    nc.sync.dma_start(
                        out[b0*P:(b+1)*P, :].rearrange("(j p) r -> p j r", p=P),
                        ost[:, 0:nb_in, :])
    return nc


def _buildc(TB):
    nc = _build(TB)
    nc.compile()
    return nc


# ---------------------------------------------------------------- entry

def kernel(feat, src, dst, W1, al1, ar1, b1, W2, al2, ar2, b2, resW2):
    import time
    in_maps, TB, newid = _host_prep(
        feat, src, dst, W1, al1, ar1, W2, al2, ar2, resW2)
    nc = _finalize(_build(TB))
    run = _prepare(nc, in_maps)
    t0 = time.perf_counter()
    res, _ = run()
    wall = time.perf_counter() - t0
    rows = np.concatenate([res[c]["out_shard"] for c in range(NC)], axis=0)
    out = rows[newid]
    _timing.update(dict(run=run, wall=wall, T=sum(TB)))
    return out.astype(np.float32)

